# revision 2
# baseline (speedup 1.0000x reference)
"""JPEG encoder Bass kernel for TRN2 — self-contained, 8-core data-parallel.

kernel(img, D, Q) -> (flatten, no_quan_flatten), matching the reference:
    per 8x8 block: dct = D @ (X - 128) @ D.T ; quant = round(dct / Q);
    both zigzag-gathered + channel-concatenated to (256, 512, 192).

Design: the whole per-block pipeline is one linear map on the flattened
64-pixel block, folded into fp32r matmuls with matrix
M = kron(D, D)[zigzag, :] (and M / q_zz for the quant path; round done as
(x + 1.5*2^23) - 1.5*2^23 on the vector engine).

Per-core (64 batches): strip DMA loads (contiguous rows) -> regroup pass
(free-dim permute (i,bw,j)->(bw,i,j), subtract 128, round to fp32r) ->
PE transposes (64-wide) to put pixels on partitions -> two fp32r matmuls
(K=128 c0|c1 stacked + K=64 c2, N=384 = [nq 192 | q-preround 192]) ->
copy/round -> contiguous (2, 64, 192) output-block DMAs.
"""

import numpy as np
import concourse.mybir as mybir
import concourse.tile as tile
from concourse import bacc
from concourse.bass_utils import run_bass_kernel_spmd
from concourse.masks import make_identity

F32 = mybir.dt.float32
F32R = mybir.dt.float32r
MAGIC = 12582912.0  # 1.5 * 2**23
P = 8
B, C, H, W = 512, 3, 128, 128
NCORES = 8
BSH = B // NCORES          # 64 batches per core
N = (H // P) * (W // P)    # 256 blocks per plane
CZ = C * P * P             # 192


def _zigzag_flat_idx(n=P):
    order = []
    for s in range(2 * n - 1):
        cells = [(r, s - r) for r in range(max(0, s - n + 1), min(s, n - 1) + 1)]
        if s % 2 == 0:
            cells.reverse()
        order.extend(cells)
    return np.array([r * n + c for r, c in order], dtype=np.int32)


def _build_rhs(D: np.ndarray, Q: np.ndarray):
    ZZ = _zigzag_flat_idx()
    D64 = D.astype(np.float64)
    KD = np.kron(D64, D64)[ZZ, :]          # (64 zz, 64 pix)
    q_zz = Q.astype(np.float64).flatten()[ZZ]
    KDq = KD / q_zz[:, None]
    Mt = KD.T.astype(np.float32)           # (64 pix, 64 zz)
    Mqt = KDq.T.astype(np.float32)
    rhs01 = np.zeros((128, 384), dtype=np.float32)
    rhs2 = np.zeros((64, 384), dtype=np.float32)
    for c, r in ((0, rhs01), (1, rhs01), (2, rhs2)):
        p0 = 64 if c == 1 else 0
        r[p0:p0 + 64, c * 64:(c + 1) * 64] = Mt
        r[p0:p0 + 64, 192 + c * 64:192 + (c + 1) * 64] = Mqt
    return rhs01, rhs2


def _build_nc():
    nc = bacc.Bacc("TRN2", target_bir_lowering=False, debug=False)

    img = nc.dram_tensor("img", [BSH, C, H, W], F32, kind="ExternalInput")
    rhs01 = nc.dram_tensor("rhs01", [128, 384], F32, kind="ExternalInput")
    rhs2 = nc.dram_tensor("rhs2", [64, 384], F32, kind="ExternalInput")
    nq = nc.dram_tensor("nq", [N, BSH, CZ], F32, kind="ExternalOutput")
    qq = nc.dram_tensor("qq", [N, BSH, CZ], F32, kind="ExternalOutput")

    AddOp = mybir.AluOpType.add
    SubOp = mybir.AluOpType.subtract
    Copy = mybir.ActivationFunctionType.Copy

    imgv = img[:].rearrange(
        "b c (bp brp i) w -> c bp brp b (i w)", brp=2, i=P
    )

    with tile.TileContext(nc) as tc:
        with (
            tc.tile_pool(name="const", bufs=1) as constp,
            tc.tile_pool(name="sload", bufs=2) as sload,
            tc.tile_pool(name="greg", bufs=2) as greg,
            tc.tile_pool(name="xt", bufs=4) as xtp,
            tc.tile_pool(name="outs", bufs=4) as outp,
            tc.tile_pool(name="psx", bufs=4, space="PSUM") as psx,
            tc.tile_pool(name="pso", bufs=4, space="PSUM") as pso,
        ):
            r01 = constp.tile([128, 384], F32)
            r2 = constp.tile([64, 384], F32)
            nc.sync.dma_start(out=r01[:], in_=rhs01[:])
            nc.sync.dma_start(out=r2[:], in_=rhs2[:])
            r01r = constp.tile([128, 384], F32R)
            r2r = constp.tile([64, 384], F32R)
            nc.vector.tensor_copy(r01r[:], r01[:])
            nc.vector.tensor_copy(r2r[:], r2[:])
            ident = constp.tile([128, 64], F32)
            make_identity(nc, ident[0:64, :])
            make_identity(nc, ident[64:128, :])
            identf = constp.tile([128, 64], F32R)
            nc.vector.tensor_copy(identf[:], ident[:])
            identr = [identf[0:64, :], identf[64:128, :]]

            for bp in range(8):  # row-block pair index
                S = [sload.tile([128, 1024], F32, tag=f"s{c}", name=f"s{c}")
                     for c in range(3)]
                for c in range(3):
                    for brp in range(2):
                        nc.sync.dma_start(
                            out=S[c][brp * 64:(brp + 1) * 64, :],
                            in_=imgv[c, bp, brp],
                        )
                G = [greg.tile([128, 1024], F32R, tag=f"g{c}", name=f"g{c}")
                     for c in range(3)]
                for c in range(3):
                    sv = S[c][:].rearrange("p (i w j) -> p w i j", i=P, w=16, j=P)
                    gv = G[c][:].rearrange("p (w i j) -> p w i j", i=P, w=16, j=P)
                    nc.vector.tensor_scalar(gv, sv, -128.0, None, AddOp)

                for brp in range(2):
                    br = bp * 2 + brp
                    stnq = outp.tile([128, 1536], F32, tag="stnq", name="stnq")
                    stq = outp.tile([128, 1536], F32, tag="stq", name="stq")
                    for bwp in range(8):
                        pxt = psx.tile([64, 384], F32R)
                        for c in range(3):
                            for k in range(2):
                                bw = bwp * 2 + k
                                nc.tensor.transpose(
                                    pxt[:, (c * 2 + k) * 64:(c * 2 + k + 1) * 64],
                                    G[c][brp * 64:(brp + 1) * 64,
                                         bw * 64:(bw + 1) * 64],
                                    identr[brp],
                                )
                        xA = xtp.tile([128, 128], F32R, tag="xa")
                        xB = xtp.tile([64, 128], F32R, tag="xb")
                        nc.scalar.activation(xA[0:64, :], pxt[:, 0:128], Copy)
                        nc.scalar.activation(xA[64:128, :], pxt[:, 128:256], Copy)
                        nc.scalar.activation(xB[:, :], pxt[:, 256:384], Copy)

                        po = pso.tile([128, 384], F32)
                        nc.tensor.matmul(po[:], xA[:], r01r[:], start=True, stop=False)
                        nc.tensor.matmul(po[:], xB[:], r2r[:], start=False, stop=True)

                        nqs = stnq[:, bwp * 192:(bwp + 1) * 192]
                        qs = stq[:, bwp * 192:(bwp + 1) * 192]
                        nc.scalar.activation(nqs, po[:, 0:192], Copy)
                        nc.vector.tensor_scalar(
                            qs, po[:, 192:384], MAGIC, MAGIC, AddOp, SubOp
                        )

                    dv_nq = nq[br * 16:(br + 1) * 16].rearrange(
                        "(bwp k) b f -> (k b) bwp f", k=2)
                    dv_qq = qq[br * 16:(br + 1) * 16].rearrange(
                        "(bwp k) b f -> (k b) bwp f", k=2)
                    nc.sync.dma_start(
                        out=dv_nq, in_=stnq[:].rearrange("p (bwp f) -> p bwp f", f=192))
                    nc.sync.dma_start(
                        out=dv_qq, in_=stq[:].rearrange("p (bwp f) -> p bwp f", f=192))

    nc.compile()
    return nc


_NC_CACHE = None


def _get_nc():
    global _NC_CACHE
    if _NC_CACHE is None:
        _NC_CACHE = _build_nc()
    return _NC_CACHE


def kernel(img, D, Q):
    img = np.ascontiguousarray(np.asarray(img, dtype=np.float32))
    D = np.asarray(D, dtype=np.float32)
    Q = np.asarray(Q, dtype=np.float32)
    rhs01, rhs2 = _build_rhs(D, Q)

    nc = _get_nc()
    in_maps = [
        {"img": img[k * BSH:(k + 1) * BSH], "rhs01": rhs01, "rhs2": rhs2}
        for k in range(NCORES)
    ]
    res = run_bass_kernel_spmd(nc, in_maps, core_ids=list(range(NCORES)))
    flatten = np.concatenate([r["qq"] for r in res.results], axis=1)
    no_quan = np.concatenate([r["nq"] for r in res.results], axis=1)
    return (flatten, no_quan)
